# revision 42
# baseline (speedup 1.0000x reference)
"""Trainium2 Bass kernel for the Actor MLP scorer (gnn_message_passing).

Computation (see reference):
    node_e  = node_embeddings[action_nodes]          # [A, 128] gather
    feats   = [node_e | region_embeddings[action_regions] | const_tail]   # [A, 1427]
    h1..h3  = relu MLP (256 wide), logits = h3 @ W4 + b4                  # [A]
    probs   = softmax(logits) over ALL actions

Strategy (8 NeuronCores, data-parallel over actions):
  - Shard A=100000 actions as 12500/core, sorted by node-id bucket
    (< 32768 vs >= 32768) so the node gather can use the int16-indexed
    DMA-gather ucode over two base-offset views of a bf16 table copy.
    Gathered rows land slot-major and are transposed to [dim, action] on
    the PE; small lead chunks prime the pipeline at startup, and the
    num_idxs registers are hoisted so the Q7 gather-library load starts
    as early as possible.
  - Layer 1 decomposition: feats @ W1 = node_e @ W1[:128]
        + onehot(region) @ (region_embeddings @ W1[128:256])
        + (tail @ W1[256:] + b1)  [host-precomputed constant bias].
    All constant projections (RPS, b1c) are computed on host.  The RPS
    lhsT is zero-padded to K=128 and W4 is replicated across 128 output
    columns so every MLP matmul uses the same 128x128 PE-array config
    (no reconfig stalls); matmul emission is batched per layer.
  - Activations stay transposed ([feature, action]); matmuls bf16 with
    fp32 PSUM; relu+bias evictions split across ScalarE/VectorE.  Gather
    PSUM->SBUF copies are emitted after each sweep's work so they queue
    behind the sweep's evictions.
  - No collectives: each core writes its logits; the global softmax
    normalization (exp/sum/divide) happens on host during unsharding.
"""

import sys

for _p in ("/opt/trn_rl_repo",):
    if _p not in sys.path:
        sys.path.insert(0, _p)

import numpy as np
import ml_dtypes
from concourse import bass, bacc, mybir, tile
from concourse import bass_utils
from concourse.masks import make_identity


# ---------------------------------------------------------------- constants
N_CORES = 8
A_FULL = 100000
N_NODES = 50000
N_REGIONS = 8
D = 128
H = 256
G = 147
IN_DIM = 2 * D + N_REGIONS * D + G          # 1427
F32 = mybir.dt.float32
BF16 = mybir.dt.bfloat16
I16 = mybir.dt.int16

A_PC = A_FULL // N_CORES                    # 12500
SPLIT = 32768                               # int16 index range boundary
C0 = 8704                                   # capacity, node id < 32768
C1 = 4608                                   # capacity, node id >= 32768
A_PAD = C0 + C1                             # 13312 = 26*512
ATILE = 512
N_AT = A_PAD // ATILE                       # 26
GCHUNK = 1024                               # idxs per dma_gather call

USE_TGATHER = False                         # dma_gather transpose mode
USE_DMAT = False                            # xbar DMA transpose (vs PE)


def _gather_chunks(total, lead=()):
    """Chunk a zone; `lead` lets the first chunks be small so the pipeline
    primes quickly at startup."""
    out, off = [], 0
    for n in lead:
        out.append((off, n))
        off += n
    while off < total:
        n = min(GCHUNK, total - off)
        out.append((off, n))
        off += n
    return out


def build_graph():
    nc = bacc.Bacc("TRN2", target_bir_lowering=False, debug=False,
                   num_devices=N_CORES, num_swdge_queues=4)

    # ---- I/O --------------------------------------------------------------
    node_emb = nc.dram_tensor("node_emb", [N_NODES, D], BF16, kind="ExternalInput")
    wa = nc.dram_tensor("wa", [D, H], BF16, kind="ExternalInput")
    w2b = nc.dram_tensor("w2b", [H, H], BF16, kind="ExternalInput")
    w3b = nc.dram_tensor("w3b", [H, H], BF16, kind="ExternalInput")
    # rps padded to 128 rows and w4 replicated across 128 columns so the
    # onehot and logit matmuls use the same full 128x128 PE config as the
    # rest (no PE-array reconfig stalls)
    rps_w = nc.dram_tensor("rps_w", [128, H], BF16, kind="ExternalInput")
    w4b = nc.dram_tensor("w4b", [128, 2 * 128], BF16, kind="ExternalInput")
    identw = nc.dram_tensor("identw", [128, 128], BF16, kind="ExternalInput")
    # cols 0:2 b1c | 2:4 b2 | 4:6 b3 | [0,6] b4
    packed = nc.dram_tensor("packed", [128, 8], F32, kind="ExternalInput")
    idx0 = nc.dram_tensor("idx0", [128, C0 // 16], I16, kind="ExternalInput")
    idx1 = nc.dram_tensor("idx1", [128, C1 // 16], I16, kind="ExternalInput")
    onehot = nc.dram_tensor("onehot", [N_REGIONS, A_PAD], BF16, kind="ExternalInput")

    out_logits = nc.dram_tensor("out_logits", [1, A_PAD], F32, kind="ExternalOutput")

    with tile.TileContext(nc) as tc:
        with (
            tc.tile_pool(name="const", bufs=1) as cpool,
            tc.tile_pool(name="hbuf", bufs=2) as hpool,
            tc.tile_pool(name="graw", bufs=6) as gpool,
            tc.tile_pool(name="pnt", bufs=1, space="PSUM") as pnt_pool,
            tc.tile_pool(name="ph", bufs=7, space="PSUM") as ph_pool,
        ):
            # ---- index loads first: gathers depend on them ---------------
            i0 = cpool.tile([128, C0 // 16], I16, tag="i0")
            nc.sync.dma_start(out=i0[:], in_=idx0[:])
            i1 = cpool.tile([128, C1 // 16], I16, tag="i1")
            nc.sync.dma_start(out=i1[:], in_=idx1[:])

            # ---- constant loads (host pre-cast bf16) ----------------------
            w1a = cpool.tile([128, H], BF16, tag="w1a")
            nc.sync.dma_start(out=w1a[:], in_=wa[:])
            rps = cpool.tile([128, H], BF16, tag="rps")
            nc.sync.dma_start(out=rps[:], in_=rps_w[:])
            pk = cpool.tile([128, 8], F32, tag="pk")
            nc.sync.dma_start(out=pk[:], in_=packed[:])
            # small 8-row onehot: keeps DMA traffic light while the Q7
            # gather library loads (a fat DMA here slows the lib fetch)
            ohs = cpool.tile([N_REGIONS, A_PAD], BF16, tag="ohs")
            nc.scalar.dma_start(out=ohs[:], in_=onehot[:])
            w2t = [cpool.tile([128, H], BF16, tag=f"w2_{k}", name=f"w2_{k}")
                   for k in range(2)]
            w3t = [cpool.tile([128, H], BF16, tag=f"w3_{k}", name=f"w3_{k}")
                   for k in range(2)]
            for k in range(2):
                nc.scalar.dma_start(out=w2t[k][:], in_=w2b[k * 128:(k + 1) * 128, :])
                nc.scalar.dma_start(out=w3t[k][:], in_=w3b[k * 128:(k + 1) * 128, :])
            w4s = cpool.tile([128, 2 * 128], BF16, tag="w4s")
            nc.sync.dma_start(out=w4s[:], in_=w4b[:])

            b1s = pk[:, 0:2]
            b2s = pk[:, 2:4]
            b3s = pk[:, 4:6]
            b4s = pk[0:1, 6:7]

            lrow = cpool.tile([1, A_PAD], F32, tag="lrow")

            # ---- node gather: nts_all[d, slot] = node_emb[id(slot), d] ---
            nts_all = cpool.tile([128, A_PAD], BF16, tag="nts_all")
            gather_plan = (
                [(0, off, n, 0)
                 for off, n in _gather_chunks(C0, lead=(256, 256, 512))]
                + [(C0, off, n, 1) for off, n in _gather_chunks(C1)])

            if not USE_TGATHER and not USE_DMAT:
                # host-provided identity: keeps the gpsimd queue clear so the
                # Q7 gather-library load starts as early as possible
                ident = cpool.tile([128, 128], BF16, tag="ident")
                nc.sync.dma_start(out=ident[:], in_=identw[:])

            # one-time register loads for the gather index counts
            nregs = {n: nc.gpsimd.to_reg(n) for n in (256, 512, 1024)}

            def emit_gather(gi):
                zone, off, n, grp = gather_plan[gi]
                gsrc = node_emb[0:SPLIT, :] if grp == 0 \
                    else node_emb[SPLIT:N_NODES, :]
                itile = i0 if grp == 0 else i1
                s0 = zone + off
                nreg = nregs[n]
                if USE_TGATHER:
                    nc.gpsimd.dma_gather(
                        out_ap=nts_all[:, s0:s0 + n].unsqueeze(1),
                        in_ap=gsrc,
                        idxs_ap=itile[:, off // 16:(off + n) // 16],
                        num_idxs=n, num_idxs_reg=nreg,
                        elem_size=D, transpose=True, single_packet=False,
                        queue_num=1)
                    return n
                graw = gpool.tile([128, n // 128, D], BF16, tag="graw",
                                  name="graw")
                nc.gpsimd.dma_gather(
                    out_ap=graw[:],
                    in_ap=gsrc,
                    idxs_ap=itile[:, off // 16:(off + n) // 16],
                    num_idxs=n, num_idxs_reg=nreg,
                    elem_size=D, transpose=False, single_packet=False,
                    queue_num=1 + (gi % 8) % 3)
                if USE_DMAT:
                    nc.sync.dma_start_transpose(
                        out=nts_all[:, s0:s0 + n].rearrange(
                            "p (c i) -> p c i", i=128),
                        in_=graw[:].rearrange("p c d -> p (c d)"))
                    return n
                nt_ps = pnt_pool.tile([128, GCHUNK], BF16, space="PSUM",
                                      tag="nt_ps", name="nt_ps")
                for c in range(n // 128):
                    nc.tensor.transpose(
                        out=nt_ps[:, c * 128:(c + 1) * 128],
                        in_=graw[:, c, :], identity=ident[:])
                if gi % 2 == 0:
                    nc.scalar.activation(
                        out=nts_all[:, s0:s0 + n], in_=nt_ps[:, 0:n],
                        func=mybir.ActivationFunctionType.Copy)
                else:
                    nc.vector.tensor_copy(out=nts_all[:, s0:s0 + n],
                                          in_=nt_ps[:, 0:n])
                return n

            def evict_relu(engine, dst, src, bias_ap):
                if engine == "act":
                    nc.scalar.activation(
                        out=dst, in_=src,
                        func=mybir.ActivationFunctionType.Relu, bias=bias_ap)
                else:
                    nc.vector.tensor_scalar(
                        out=dst, in0=src, scalar1=bias_ap, scalar2=0.0,
                        op0=mybir.AluOpType.add, op1=mybir.AluOpType.max)

            # ---- main loop: sweeps of 2 action tiles ----------------------
            SWEEP = 2
            t0s = list(range(0, N_AT, SWEEP))
            out_done = 0                       # cols already DMAed out

            def flush_logits(upto):
                nonlocal out_done
                if upto > out_done:
                    nc.sync.dma_start(out=out_logits[0:1, out_done:upto],
                                      in_=lrow[0:1, out_done:upto])
                    out_done = upto

            gi_next = 0
            covered = 0
            # prime: cover the first two sweeps plus one chunk of lookahead
            while gi_next < len(gather_plan) and \
                    covered < 2 * SWEEP * ATILE + GCHUNK:
                covered += emit_gather(gi_next)
                gi_next += 1
            for si, t0 in enumerate(t0s):
                tiles = list(range(t0, min(t0 + SWEEP, N_AT)))
                sls = [slice(t * ATILE, (t + 1) * ATILE) for t in tiles]
                nt = len(tiles)

                # layer 1: all node matmuls, then all onehot matmuls, so the
                # PE array config (128x128 vs 8x128) switches once per sweep
                h1 = [[hpool.tile([128, ATILE], BF16, tag=f"h1_{j}_{i}",
                                  name=f"h1_{j}_{i}")
                       for j in range(2)] for i in range(nt)]
                hps1 = [[ph_pool.tile([128, ATILE], F32, space="PSUM",
                                      tag="hps", name="hps")
                         for _ in range(nt)] for _ in range(2)]
                for j in range(2):
                    for i in range(nt):
                        nc.tensor.matmul(out=hps1[j][i][:],
                                         lhsT=w1a[:, j * 128:(j + 1) * 128],
                                         rhs=nts_all[:, sls[i]],
                                         start=True, stop=False)
                for j in range(2):
                    for i in range(nt):
                        nc.tensor.matmul(out=hps1[j][i][:],
                                         lhsT=rps[0:N_REGIONS,
                                                  j * 128:(j + 1) * 128],
                                         rhs=ohs[0:N_REGIONS, sls[i]],
                                         start=False, stop=True)
                for j in range(2):
                    for i in range(nt):
                        evict_relu("act" if (i + j) % 2 == 0 else "dve",
                                   h1[i][j][:], hps1[j][i][:], b1s[:, j:j + 1])

                # layers 2 and 3
                hin = h1
                for li, (wt, bs) in enumerate(((w2t, b2s), (w3t, b3s))):
                    hout = [[hpool.tile([128, ATILE], BF16,
                                        tag=f"h{li + 2}_{j}_{i}",
                                        name=f"h{li + 2}_{j}_{i}")
                             for j in range(2)] for i in range(nt)]
                    for j in range(2):
                        hps = [ph_pool.tile([128, ATILE], F32, space="PSUM",
                                            tag="hps", name="hps")
                               for _ in range(nt)]
                        for k in range(2):
                            for i in range(nt):
                                nc.tensor.matmul(
                                    out=hps[i][:],
                                    lhsT=wt[k][:, j * 128:(j + 1) * 128],
                                    rhs=hin[i][k][:],
                                    start=(k == 0), stop=(k == 1))
                        for i in range(nt):
                            evict_relu("act" if (i + j + li) % 2 == 0 else "dve",
                                       hout[i][j][:], hps[i][:], bs[:, j:j + 1])
                    hin = hout

                # layer 4: logits.  w4 replicated across 128 output columns
                # keeps the full 128x128 PE config (no reconfig stall); the
                # eviction reads row 0 of the (identical-row) PSUM result.
                lgs = [ph_pool.tile([128, ATILE], F32, space="PSUM", tag="hps",
                                    name="hps") for _ in range(nt)]
                for k in range(2):
                    for i in range(nt):
                        nc.tensor.matmul(out=lgs[i][:],
                                         lhsT=w4s[:, k * 128:(k + 1) * 128],
                                         rhs=hin[i][k][:],
                                         start=(k == 0), stop=(k == 1))
                for i in range(nt):
                    if i % 2 == 0:
                        nc.scalar.activation(
                            out=lrow[0:1, sls[i]], in_=lgs[i][0:1, :],
                            func=mybir.ActivationFunctionType.Identity,
                            bias=b4s)
                    else:
                        nc.vector.tensor_scalar_add(
                            out=lrow[0:1, sls[i]], in0=lgs[i][0:1, :],
                            scalar1=b4s)
                if si in (3, 6, 9, 11):
                    flush_logits((t0 + SWEEP) * ATILE)
                # gathers for upcoming sweeps — emitted AFTER this sweep's
                # work so their PSUM->SBUF copies queue behind this sweep's
                # evictions on the Scalar/Vector engines
                nxt = min(t0 + 2 * SWEEP, N_AT) * ATILE
                while gi_next < len(gather_plan) and covered < nxt + GCHUNK:
                    covered += emit_gather(gi_next)
                    gi_next += 1

            flush_logits(A_PAD)

    nc.compile()
    return nc


_GRAPH_CACHE = {}


def _get_graph():
    if "g" not in _GRAPH_CACHE:
        _GRAPH_CACHE["g"] = build_graph()
    return _GRAPH_CACHE["g"]


def _wrap_idx(ix):
    """int16 index layout for dma_gather: [16, N/16] column-wrapped,
    replicated 8x down the partitions."""
    w = ix.reshape(-1, 16).T
    return np.ascontiguousarray(np.tile(w, (8, 1)))


def make_in_maps(node_embeddings, region_embeddings, global_context,
                 W1, b1, W2, b2, W3, b3, W4, b4,
                 action_nodes, action_regions):
    """Host-side sharding / marshalling. Returns (in_maps, per-core metas)."""
    W1 = np.asarray(W1, np.float32)
    an = np.asarray(action_nodes).astype(np.int64)
    ar = np.asarray(action_regions).astype(np.int64)
    node_bf16 = np.ascontiguousarray(
        np.asarray(node_embeddings, np.float32).astype(ml_dtypes.bfloat16))
    region_embeddings = np.asarray(region_embeddings, np.float32)

    tail = np.concatenate([
        region_embeddings.reshape(-1),
        np.asarray(global_context, np.float32).reshape(-1)])
    b1c = (np.asarray(b1, np.float32)
           + tail @ W1[2 * D:IN_DIM, :]).astype(np.float32)   # [256]
    rps_np = np.zeros((128, H), ml_dtypes.bfloat16)
    rps_np[0:N_REGIONS] = (region_embeddings @ W1[D:2 * D, :]
                           ).astype(ml_dtypes.bfloat16)
    wa_np = np.ascontiguousarray(W1[0:D, :].astype(ml_dtypes.bfloat16))
    w2b_np = np.ascontiguousarray(
        np.asarray(W2, np.float32).astype(ml_dtypes.bfloat16))
    w3b_np = np.ascontiguousarray(
        np.asarray(W3, np.float32).astype(ml_dtypes.bfloat16))
    w4f = np.asarray(W4, np.float32).reshape(-1)
    w4b_np = np.ascontiguousarray(np.concatenate(
        [np.repeat(w4f[k * 128:(k + 1) * 128, None], 128, axis=1)
         for k in range(2)], axis=1).astype(ml_dtypes.bfloat16))

    pk_base = np.zeros((128, 8), np.float32)
    pk_base[:, 0:2] = b1c.reshape(2, 128).T
    pk_base[:, 2:4] = np.asarray(b2, np.float32).reshape(2, 128).T
    pk_base[:, 4:6] = np.asarray(b3, np.float32).reshape(2, 128).T
    pk_base[0, 6] = np.asarray(b4, np.float32).reshape(-1)[0]

    in_maps, metas = [], []
    for c in range(N_CORES):
        s = c * A_PC
        nodes = an[s:s + A_PC]
        regions = ar[s:s + A_PC]
        grp = (nodes >= SPLIT).astype(np.int8)
        order = np.argsort(grp, kind="stable")      # group0 first, stable
        c0 = int((grp == 0).sum())
        c1 = A_PC - c0
        if c0 > C0 or c1 > C1:
            raise RuntimeError(
                f"core {c}: group sizes {c0}/{c1} exceed capacities {C0}/{C1}")
        sn = nodes[order]
        sr = regions[order]

        ix0 = np.zeros(C0, np.int16)
        ix0[:c0] = sn[:c0].astype(np.int16)
        ix1 = np.zeros(C1, np.int16)
        ix1[:c1] = (sn[c0:] - SPLIT).astype(np.int16)

        slots = np.concatenate([np.arange(c0), C0 + np.arange(c1)])
        oh = np.zeros((N_REGIONS, A_PAD), ml_dtypes.bfloat16)
        oh[sr, slots] = 1.0

        in_maps.append({
            "node_emb": node_bf16,
            "wa": wa_np, "w2b": w2b_np, "w3b": w3b_np,
            "rps_w": rps_np, "w4b": w4b_np,
            "identw": np.eye(128, dtype=ml_dtypes.bfloat16),
            "packed": pk_base,
            "idx0": _wrap_idx(ix0), "idx1": _wrap_idx(ix1),
            "onehot": oh,
        })
        metas.append((order, slots))
    return in_maps, metas


def _unshard(results, metas):
    logits = np.empty(A_FULL, np.float32)
    for c in range(N_CORES):
        order, slots = metas[c]
        lg = np.asarray(results[c]).reshape(-1)[slots]
        logits[c * A_PC:(c + 1) * A_PC][order] = lg
    le = logits.astype(np.float64)
    e = np.exp(le - le.max())
    probs = (e / e.sum()).astype(np.float32)
    return probs, logits


def kernel(**inputs):
    nc = _get_graph()
    in_maps, metas = make_in_maps(**inputs)
    res = bass_utils.run_bass_kernel_spmd(
        nc, in_maps, core_ids=list(range(N_CORES)))
    return _unshard([res.results[c]["out_logits"] for c in range(N_CORES)],
                    metas)


# revision 43
# speedup vs baseline: 1.1667x; 1.1667x over previous
"""Trainium2 Bass kernel for the Actor MLP scorer (gnn_message_passing).

Computation (see reference):
    node_e  = node_embeddings[action_nodes]          # [A, 128] gather
    feats   = [node_e | region_embeddings[action_regions] | const_tail]   # [A, 1427]
    h1..h3  = relu MLP (256 wide), logits = h3 @ W4 + b4                  # [A]
    probs   = softmax(logits) over ALL actions

Strategy (8 NeuronCores, data-parallel over actions):
  - Shard A=100000 actions as 12500/core, sorted by node-id bucket
    (< 32768 vs >= 32768) so the node gather can use the int16-indexed
    DMA-gather ucode over two base-offset views of a bf16 table copy.
    Gathered rows land slot-major and are transposed to [dim, action] on
    the PE; small lead chunks prime the pipeline at startup, and the
    num_idxs registers are hoisted so the Q7 gather-library load starts
    as early as possible.
  - Layer 1 decomposition: feats @ W1 = node_e @ W1[:128]
        + onehot(region) @ (region_embeddings @ W1[128:256])
        + (tail @ W1[256:] + b1)  [host-precomputed constant bias].
    All constant projections (RPS, b1c) are computed on host.  The RPS
    lhsT is zero-padded to K=128 and W4 is replicated across 128 output
    columns so every MLP matmul uses the same 128x128 PE-array config
    (no reconfig stalls); matmul emission is batched per layer.
  - Activations stay transposed ([feature, action]); matmuls bf16 with
    fp32 PSUM; relu+bias evictions split across ScalarE/VectorE.  Gather
    PSUM->SBUF copies are emitted after each sweep's work so they queue
    behind the sweep's evictions.
  - No collectives: each core writes its logits; the global softmax
    normalization (exp/sum/divide) happens on host during unsharding.
"""

import sys

for _p in ("/opt/trn_rl_repo",):
    if _p not in sys.path:
        sys.path.insert(0, _p)

import numpy as np
import ml_dtypes
from concourse import bass, bacc, mybir, tile
from concourse import bass_utils
from concourse.masks import make_identity


# ---------------------------------------------------------------- constants
N_CORES = 8
A_FULL = 100000
N_NODES = 50000
N_REGIONS = 8
D = 128
H = 256
G = 147
IN_DIM = 2 * D + N_REGIONS * D + G          # 1427
F32 = mybir.dt.float32
BF16 = mybir.dt.bfloat16
I16 = mybir.dt.int16

A_PC = A_FULL // N_CORES                    # 12500
SPLIT = 32768                               # int16 index range boundary
C0 = 8704                                   # capacity, node id < 32768
C1 = 4608                                   # capacity, node id >= 32768
A_PAD = C0 + C1                             # 13312 = 26*512
ATILE = 512
N_AT = A_PAD // ATILE                       # 26
GCHUNK = 1024                               # idxs per dma_gather call

USE_TGATHER = False                         # dma_gather transpose mode
USE_DMAT = False                            # xbar DMA transpose (vs PE)


def _gather_chunks(total, lead=()):
    """Chunk a zone; `lead` lets the first chunks be small so the pipeline
    primes quickly at startup."""
    out, off = [], 0
    for n in lead:
        out.append((off, n))
        off += n
    while off < total:
        n = min(GCHUNK, total - off)
        out.append((off, n))
        off += n
    return out


def build_graph():
    nc = bacc.Bacc("TRN2", target_bir_lowering=False, debug=False,
                   num_devices=N_CORES, num_swdge_queues=4)

    # ---- I/O --------------------------------------------------------------
    node_emb = nc.dram_tensor("node_emb", [N_NODES, D], BF16, kind="ExternalInput")
    wa = nc.dram_tensor("wa", [D, H], BF16, kind="ExternalInput")
    w2b = nc.dram_tensor("w2b", [H, H], BF16, kind="ExternalInput")
    w3b = nc.dram_tensor("w3b", [H, H], BF16, kind="ExternalInput")
    # rps padded to 128 rows and w4 replicated across 128 columns so the
    # onehot and logit matmuls use the same full 128x128 PE config as the
    # rest (no PE-array reconfig stalls)
    rps_w = nc.dram_tensor("rps_w", [128, H], BF16, kind="ExternalInput")
    w4b = nc.dram_tensor("w4b", [128, 2 * 128], BF16, kind="ExternalInput")
    identw = nc.dram_tensor("identw", [128, 128], BF16, kind="ExternalInput")
    # cols 0:2 b1c | 2:4 b2 | 4:6 b3 | [0,6] b4
    packed = nc.dram_tensor("packed", [128, 8], F32, kind="ExternalInput")
    idx0 = nc.dram_tensor("idx0", [128, C0 // 16], I16, kind="ExternalInput")
    idx1 = nc.dram_tensor("idx1", [128, C1 // 16], I16, kind="ExternalInput")
    onehot = nc.dram_tensor("onehot", [128, A_PAD], BF16, kind="ExternalInput")

    out_logits = nc.dram_tensor("out_logits", [1, A_PAD], F32, kind="ExternalOutput")

    with tile.TileContext(nc) as tc:
        with (
            tc.tile_pool(name="const", bufs=1) as cpool,
            tc.tile_pool(name="hbuf", bufs=2) as hpool,
            tc.tile_pool(name="graw", bufs=6) as gpool,
            tc.tile_pool(name="pnt", bufs=1, space="PSUM") as pnt_pool,
            tc.tile_pool(name="ph", bufs=7, space="PSUM") as ph_pool,
        ):
            # ---- index loads first: gathers depend on them ---------------
            i0 = cpool.tile([128, C0 // 16], I16, tag="i0")
            nc.sync.dma_start(out=i0[:], in_=idx0[:])
            i1 = cpool.tile([128, C1 // 16], I16, tag="i1")
            nc.sync.dma_start(out=i1[:], in_=idx1[:])

            # ---- constant loads (host pre-cast bf16) ----------------------
            w1a = cpool.tile([128, H], BF16, tag="w1a")
            nc.sync.dma_start(out=w1a[:], in_=wa[:])
            rps = cpool.tile([128, H], BF16, tag="rps")
            nc.sync.dma_start(out=rps[:], in_=rps_w[:])
            pk = cpool.tile([128, 8], F32, tag="pk")
            nc.sync.dma_start(out=pk[:], in_=packed[:])
            # onehot quarters so early action tiles' columns land first
            ohs = cpool.tile([128, A_PAD], BF16, tag="ohs")
            OHQ = A_PAD // 4
            for q in range(4):
                nc.scalar.dma_start(out=ohs[:, q * OHQ:(q + 1) * OHQ],
                                    in_=onehot[:, q * OHQ:(q + 1) * OHQ])
            w2t = [cpool.tile([128, H], BF16, tag=f"w2_{k}", name=f"w2_{k}")
                   for k in range(2)]
            w3t = [cpool.tile([128, H], BF16, tag=f"w3_{k}", name=f"w3_{k}")
                   for k in range(2)]
            for k in range(2):
                nc.scalar.dma_start(out=w2t[k][:], in_=w2b[k * 128:(k + 1) * 128, :])
                nc.scalar.dma_start(out=w3t[k][:], in_=w3b[k * 128:(k + 1) * 128, :])
            w4s = cpool.tile([128, 2 * 128], BF16, tag="w4s")
            nc.sync.dma_start(out=w4s[:], in_=w4b[:])

            b1s = pk[:, 0:2]
            b2s = pk[:, 2:4]
            b3s = pk[:, 4:6]
            b4s = pk[0:1, 6:7]

            lrow = cpool.tile([1, A_PAD], F32, tag="lrow")

            # ---- node gather: nts_all[d, slot] = node_emb[id(slot), d] ---
            nts_all = cpool.tile([128, A_PAD], BF16, tag="nts_all")
            gather_plan = (
                [(0, off, n, 0)
                 for off, n in _gather_chunks(C0, lead=(256, 256, 512))]
                + [(C0, off, n, 1) for off, n in _gather_chunks(C1)])

            if not USE_TGATHER and not USE_DMAT:
                # host-provided identity: keeps the gpsimd queue clear so the
                # Q7 gather-library load starts as early as possible
                ident = cpool.tile([128, 128], BF16, tag="ident")
                nc.sync.dma_start(out=ident[:], in_=identw[:])

            # one-time register loads for the gather index counts
            nregs = {n: nc.gpsimd.to_reg(n) for n in (256, 512, 1024)}

            def emit_gather(gi):
                zone, off, n, grp = gather_plan[gi]
                gsrc = node_emb[0:SPLIT, :] if grp == 0 \
                    else node_emb[SPLIT:N_NODES, :]
                itile = i0 if grp == 0 else i1
                s0 = zone + off
                nreg = nregs[n]
                if USE_TGATHER:
                    nc.gpsimd.dma_gather(
                        out_ap=nts_all[:, s0:s0 + n].unsqueeze(1),
                        in_ap=gsrc,
                        idxs_ap=itile[:, off // 16:(off + n) // 16],
                        num_idxs=n, num_idxs_reg=nreg,
                        elem_size=D, transpose=True, single_packet=False,
                        queue_num=1)
                    return n
                graw = gpool.tile([128, n // 128, D], BF16, tag="graw",
                                  name="graw")
                nc.gpsimd.dma_gather(
                    out_ap=graw[:],
                    in_ap=gsrc,
                    idxs_ap=itile[:, off // 16:(off + n) // 16],
                    num_idxs=n, num_idxs_reg=nreg,
                    elem_size=D, transpose=False, single_packet=False,
                    queue_num=1 + (gi % 8) % 3)
                if USE_DMAT:
                    nc.sync.dma_start_transpose(
                        out=nts_all[:, s0:s0 + n].rearrange(
                            "p (c i) -> p c i", i=128),
                        in_=graw[:].rearrange("p c d -> p (c d)"))
                    return n
                nt_ps = pnt_pool.tile([128, GCHUNK], BF16, space="PSUM",
                                      tag="nt_ps", name="nt_ps")
                for c in range(n // 128):
                    nc.tensor.transpose(
                        out=nt_ps[:, c * 128:(c + 1) * 128],
                        in_=graw[:, c, :], identity=ident[:])
                if gi % 2 == 0:
                    nc.scalar.activation(
                        out=nts_all[:, s0:s0 + n], in_=nt_ps[:, 0:n],
                        func=mybir.ActivationFunctionType.Copy)
                else:
                    nc.vector.tensor_copy(out=nts_all[:, s0:s0 + n],
                                          in_=nt_ps[:, 0:n])
                return n

            def evict_relu(engine, dst, src, bias_ap):
                if engine == "act":
                    nc.scalar.activation(
                        out=dst, in_=src,
                        func=mybir.ActivationFunctionType.Relu, bias=bias_ap)
                else:
                    nc.vector.tensor_scalar(
                        out=dst, in0=src, scalar1=bias_ap, scalar2=0.0,
                        op0=mybir.AluOpType.add, op1=mybir.AluOpType.max)

            # ---- main loop: sweeps of 2 action tiles ----------------------
            SWEEP = 2
            t0s = list(range(0, N_AT, SWEEP))
            out_done = 0                       # cols already DMAed out

            def flush_logits(upto):
                nonlocal out_done
                if upto > out_done:
                    nc.sync.dma_start(out=out_logits[0:1, out_done:upto],
                                      in_=lrow[0:1, out_done:upto])
                    out_done = upto

            gi_next = 0
            covered = 0
            # prime: cover the first two sweeps plus one chunk of lookahead
            while gi_next < len(gather_plan) and \
                    covered < 2 * SWEEP * ATILE + GCHUNK:
                covered += emit_gather(gi_next)
                gi_next += 1
            for si, t0 in enumerate(t0s):
                tiles = list(range(t0, min(t0 + SWEEP, N_AT)))
                sls = [slice(t * ATILE, (t + 1) * ATILE) for t in tiles]
                nt = len(tiles)

                # layer 1: all node matmuls, then all onehot matmuls, so the
                # PE array config (128x128 vs 8x128) switches once per sweep
                h1 = [[hpool.tile([128, ATILE], BF16, tag=f"h1_{j}_{i}",
                                  name=f"h1_{j}_{i}")
                       for j in range(2)] for i in range(nt)]
                hps1 = [[ph_pool.tile([128, ATILE], F32, space="PSUM",
                                      tag="hps", name="hps")
                         for _ in range(nt)] for _ in range(2)]
                for j in range(2):
                    for i in range(nt):
                        nc.tensor.matmul(out=hps1[j][i][:],
                                         lhsT=w1a[:, j * 128:(j + 1) * 128],
                                         rhs=nts_all[:, sls[i]],
                                         start=True, stop=False)
                for j in range(2):
                    for i in range(nt):
                        nc.tensor.matmul(out=hps1[j][i][:],
                                         lhsT=rps[:, j * 128:(j + 1) * 128],
                                         rhs=ohs[:, sls[i]],
                                         start=False, stop=True)
                for j in range(2):
                    for i in range(nt):
                        evict_relu("act" if (i + j) % 2 == 0 else "dve",
                                   h1[i][j][:], hps1[j][i][:], b1s[:, j:j + 1])

                # layers 2 and 3
                hin = h1
                for li, (wt, bs) in enumerate(((w2t, b2s), (w3t, b3s))):
                    hout = [[hpool.tile([128, ATILE], BF16,
                                        tag=f"h{li + 2}_{j}_{i}",
                                        name=f"h{li + 2}_{j}_{i}")
                             for j in range(2)] for i in range(nt)]
                    for j in range(2):
                        hps = [ph_pool.tile([128, ATILE], F32, space="PSUM",
                                            tag="hps", name="hps")
                               for _ in range(nt)]
                        for k in range(2):
                            for i in range(nt):
                                nc.tensor.matmul(
                                    out=hps[i][:],
                                    lhsT=wt[k][:, j * 128:(j + 1) * 128],
                                    rhs=hin[i][k][:],
                                    start=(k == 0), stop=(k == 1))
                        for i in range(nt):
                            evict_relu("act" if (i + j + li) % 2 == 0 else "dve",
                                       hout[i][j][:], hps[i][:], bs[:, j:j + 1])
                    hin = hout

                # layer 4: logits.  w4 replicated across 128 output columns
                # keeps the full 128x128 PE config (no reconfig stall); the
                # eviction reads row 0 of the (identical-row) PSUM result.
                lgs = [ph_pool.tile([128, ATILE], F32, space="PSUM", tag="hps",
                                    name="hps") for _ in range(nt)]
                for k in range(2):
                    for i in range(nt):
                        nc.tensor.matmul(out=lgs[i][:],
                                         lhsT=w4s[:, k * 128:(k + 1) * 128],
                                         rhs=hin[i][k][:],
                                         start=(k == 0), stop=(k == 1))
                for i in range(nt):
                    if i % 2 == 0:
                        nc.scalar.activation(
                            out=lrow[0:1, sls[i]], in_=lgs[i][0:1, :],
                            func=mybir.ActivationFunctionType.Identity,
                            bias=b4s)
                    else:
                        nc.vector.tensor_scalar_add(
                            out=lrow[0:1, sls[i]], in0=lgs[i][0:1, :],
                            scalar1=b4s)
                if si in (3, 6, 9):
                    flush_logits((t0 + SWEEP) * ATILE)
                # gathers for upcoming sweeps — emitted AFTER this sweep's
                # work so their PSUM->SBUF copies queue behind this sweep's
                # evictions on the Scalar/Vector engines
                nxt = min(t0 + 2 * SWEEP, N_AT) * ATILE
                while gi_next < len(gather_plan) and covered < nxt + GCHUNK:
                    covered += emit_gather(gi_next)
                    gi_next += 1

            flush_logits(A_PAD)

    nc.compile()
    return nc


_GRAPH_CACHE = {}


def _get_graph():
    if "g" not in _GRAPH_CACHE:
        _GRAPH_CACHE["g"] = build_graph()
    return _GRAPH_CACHE["g"]


def _wrap_idx(ix):
    """int16 index layout for dma_gather: [16, N/16] column-wrapped,
    replicated 8x down the partitions."""
    w = ix.reshape(-1, 16).T
    return np.ascontiguousarray(np.tile(w, (8, 1)))


def make_in_maps(node_embeddings, region_embeddings, global_context,
                 W1, b1, W2, b2, W3, b3, W4, b4,
                 action_nodes, action_regions):
    """Host-side sharding / marshalling. Returns (in_maps, per-core metas)."""
    W1 = np.asarray(W1, np.float32)
    an = np.asarray(action_nodes).astype(np.int64)
    ar = np.asarray(action_regions).astype(np.int64)
    node_bf16 = np.ascontiguousarray(
        np.asarray(node_embeddings, np.float32).astype(ml_dtypes.bfloat16))
    region_embeddings = np.asarray(region_embeddings, np.float32)

    tail = np.concatenate([
        region_embeddings.reshape(-1),
        np.asarray(global_context, np.float32).reshape(-1)])
    b1c = (np.asarray(b1, np.float32)
           + tail @ W1[2 * D:IN_DIM, :]).astype(np.float32)   # [256]
    rps_np = np.zeros((128, H), ml_dtypes.bfloat16)
    rps_np[0:N_REGIONS] = (region_embeddings @ W1[D:2 * D, :]
                           ).astype(ml_dtypes.bfloat16)
    wa_np = np.ascontiguousarray(W1[0:D, :].astype(ml_dtypes.bfloat16))
    w2b_np = np.ascontiguousarray(
        np.asarray(W2, np.float32).astype(ml_dtypes.bfloat16))
    w3b_np = np.ascontiguousarray(
        np.asarray(W3, np.float32).astype(ml_dtypes.bfloat16))
    w4f = np.asarray(W4, np.float32).reshape(-1)
    w4b_np = np.ascontiguousarray(np.concatenate(
        [np.repeat(w4f[k * 128:(k + 1) * 128, None], 128, axis=1)
         for k in range(2)], axis=1).astype(ml_dtypes.bfloat16))

    pk_base = np.zeros((128, 8), np.float32)
    pk_base[:, 0:2] = b1c.reshape(2, 128).T
    pk_base[:, 2:4] = np.asarray(b2, np.float32).reshape(2, 128).T
    pk_base[:, 4:6] = np.asarray(b3, np.float32).reshape(2, 128).T
    pk_base[0, 6] = np.asarray(b4, np.float32).reshape(-1)[0]

    in_maps, metas = [], []
    for c in range(N_CORES):
        s = c * A_PC
        nodes = an[s:s + A_PC]
        regions = ar[s:s + A_PC]
        grp = (nodes >= SPLIT).astype(np.int8)
        order = np.argsort(grp, kind="stable")      # group0 first, stable
        c0 = int((grp == 0).sum())
        c1 = A_PC - c0
        if c0 > C0 or c1 > C1:
            raise RuntimeError(
                f"core {c}: group sizes {c0}/{c1} exceed capacities {C0}/{C1}")
        sn = nodes[order]
        sr = regions[order]

        ix0 = np.zeros(C0, np.int16)
        ix0[:c0] = sn[:c0].astype(np.int16)
        ix1 = np.zeros(C1, np.int16)
        ix1[:c1] = (sn[c0:] - SPLIT).astype(np.int16)

        slots = np.concatenate([np.arange(c0), C0 + np.arange(c1)])
        oh = np.zeros((128, A_PAD), ml_dtypes.bfloat16)
        oh[sr, slots] = 1.0

        in_maps.append({
            "node_emb": node_bf16,
            "wa": wa_np, "w2b": w2b_np, "w3b": w3b_np,
            "rps_w": rps_np, "w4b": w4b_np,
            "identw": np.eye(128, dtype=ml_dtypes.bfloat16),
            "packed": pk_base,
            "idx0": _wrap_idx(ix0), "idx1": _wrap_idx(ix1),
            "onehot": oh,
        })
        metas.append((order, slots))
    return in_maps, metas


def _unshard(results, metas):
    logits = np.empty(A_FULL, np.float32)
    for c in range(N_CORES):
        order, slots = metas[c]
        lg = np.asarray(results[c]).reshape(-1)[slots]
        logits[c * A_PC:(c + 1) * A_PC][order] = lg
    le = logits.astype(np.float64)
    e = np.exp(le - le.max())
    probs = (e / e.sum()).astype(np.float32)
    return probs, logits


def kernel(**inputs):
    nc = _get_graph()
    in_maps, metas = make_in_maps(**inputs)
    res = bass_utils.run_bass_kernel_spmd(
        nc, in_maps, core_ids=list(range(N_CORES)))
    return _unshard([res.results[c]["out_logits"] for c in range(N_CORES)],
                    metas)


# revision 45
# speedup vs baseline: 1.2322x; 1.0561x over previous
"""Trainium2 Bass kernel for the Actor MLP scorer (gnn_message_passing).

Computation (see reference):
    node_e  = node_embeddings[action_nodes]          # [A, 128] gather
    feats   = [node_e | region_embeddings[action_regions] | const_tail]   # [A, 1427]
    h1..h3  = relu MLP (256 wide), logits = h3 @ W4 + b4                  # [A]
    probs   = softmax(logits) over ALL actions

Strategy (8 NeuronCores, data-parallel over actions):
  - Shard A=100000 actions as 12500/core, sorted by node-id bucket
    (< 32768 vs >= 32768) so the node gather can use the int16-indexed
    DMA-gather ucode over two base-offset views of a bf16 table copy.
    Gathered rows land slot-major and are transposed to [dim, action] on
    the PE; small lead chunks prime the pipeline at startup, and the
    num_idxs registers are hoisted so the Q7 gather-library load starts
    as early as possible.
  - Layer 1 decomposition: feats @ W1 = node_e @ W1[:128]
        + onehot(region) @ (region_embeddings @ W1[128:256])
        + (tail @ W1[256:] + b1)  [host-precomputed constant bias].
    All constant projections (RPS, b1c) are computed on host.  The RPS
    lhsT is zero-padded to K=128 and W4 is replicated across 128 output
    columns so every MLP matmul uses the same 128x128 PE-array config
    (no reconfig stalls); matmul emission is batched per layer.
  - Activations stay transposed ([feature, action]); matmuls bf16 with
    fp32 PSUM; relu+bias evictions split across ScalarE/VectorE.  Gather
    PSUM->SBUF copies are emitted after each sweep's work so they queue
    behind the sweep's evictions.
  - No collectives: each core writes its logits; the global softmax
    normalization (exp/sum/divide) happens on host during unsharding.
"""

import sys

for _p in ("/opt/trn_rl_repo",):
    if _p not in sys.path:
        sys.path.insert(0, _p)

import numpy as np
import ml_dtypes
from concourse import bass, bacc, mybir, tile
from concourse import bass_utils
from concourse.masks import make_identity


# ---------------------------------------------------------------- constants
N_CORES = 8
A_FULL = 100000
N_NODES = 50000
N_REGIONS = 8
D = 128
H = 256
G = 147
IN_DIM = 2 * D + N_REGIONS * D + G          # 1427
F32 = mybir.dt.float32
BF16 = mybir.dt.bfloat16
I16 = mybir.dt.int16

A_PC = A_FULL // N_CORES                    # 12500
SPLIT = 32768                               # int16 index range boundary
C0 = 8704                                   # capacity, node id < 32768
C1 = 4608                                   # capacity, node id >= 32768
A_PAD = C0 + C1                             # 13312 = 26*512
ATILE = 512
N_AT = A_PAD // ATILE                       # 26
GCHUNK = 1024                               # idxs per dma_gather call

USE_TGATHER = False                         # dma_gather transpose mode
USE_DMAT = False                            # xbar DMA transpose (vs PE)


def _gather_chunks(total, lead=()):
    """Chunk a zone; `lead` lets the first chunks be small so the pipeline
    primes quickly at startup."""
    out, off = [], 0
    for n in lead:
        out.append((off, n))
        off += n
    while off < total:
        n = min(GCHUNK, total - off)
        out.append((off, n))
        off += n
    return out


def build_graph():
    nc = bacc.Bacc("TRN2", target_bir_lowering=False, debug=False,
                   num_devices=N_CORES, num_swdge_queues=4)

    # ---- I/O --------------------------------------------------------------
    node_emb = nc.dram_tensor("node_emb", [N_NODES, D], BF16, kind="ExternalInput")
    wa = nc.dram_tensor("wa", [D, H], BF16, kind="ExternalInput")
    w2b = nc.dram_tensor("w2b", [H, H], BF16, kind="ExternalInput")
    w3b = nc.dram_tensor("w3b", [H, H], BF16, kind="ExternalInput")
    # rps padded to 128 rows and w4 replicated across 128 columns so the
    # onehot and logit matmuls use the same full 128x128 PE config as the
    # rest (no PE-array reconfig stalls)
    rps_w = nc.dram_tensor("rps_w", [128, H], BF16, kind="ExternalInput")
    w4b = nc.dram_tensor("w4b", [128, 2 * 128], BF16, kind="ExternalInput")
    identw = nc.dram_tensor("identw", [128, 128], BF16, kind="ExternalInput")
    # cols 0:2 b1c | 2:4 b2 | 4:6 b3 | [0,6] b4
    packed = nc.dram_tensor("packed", [128, 8], F32, kind="ExternalInput")
    idx0 = nc.dram_tensor("idx0", [128, C0 // 16], I16, kind="ExternalInput")
    idx1 = nc.dram_tensor("idx1", [128, C1 // 16], I16, kind="ExternalInput")
    onehot = nc.dram_tensor("onehot", [128, A_PAD], BF16, kind="ExternalInput")

    out_logits = nc.dram_tensor("out_logits", [1, A_PAD], F32, kind="ExternalOutput")

    with tile.TileContext(nc) as tc:
        with (
            tc.tile_pool(name="const", bufs=1) as cpool,
            tc.tile_pool(name="hbuf", bufs=2) as hpool,
            tc.tile_pool(name="graw", bufs=6) as gpool,
            tc.tile_pool(name="pnt", bufs=1, space="PSUM") as pnt_pool,
            tc.tile_pool(name="ph", bufs=7, space="PSUM") as ph_pool,
        ):
            # ---- index loads first: gathers depend on them ---------------
            i0 = cpool.tile([128, C0 // 16], I16, tag="i0")
            nc.sync.dma_start(out=i0[:], in_=idx0[:])
            i1 = cpool.tile([128, C1 // 16], I16, tag="i1")
            nc.sync.dma_start(out=i1[:], in_=idx1[:])

            # ---- constant loads (host pre-cast bf16) ----------------------
            w1a = cpool.tile([128, H], BF16, tag="w1a")
            nc.sync.dma_start(out=w1a[:], in_=wa[:])
            rps = cpool.tile([128, H], BF16, tag="rps")
            nc.sync.dma_start(out=rps[:], in_=rps_w[:])
            pk = cpool.tile([128, 8], F32, tag="pk")
            nc.sync.dma_start(out=pk[:], in_=packed[:])
            # onehot quarters so early action tiles' columns land first
            ohs = cpool.tile([128, A_PAD], BF16, tag="ohs")
            OHQ = A_PAD // 4
            for q in range(4):
                nc.scalar.dma_start(out=ohs[:, q * OHQ:(q + 1) * OHQ],
                                    in_=onehot[:, q * OHQ:(q + 1) * OHQ])
            w2t = [cpool.tile([128, H], BF16, tag=f"w2_{k}", name=f"w2_{k}")
                   for k in range(2)]
            w3t = [cpool.tile([128, H], BF16, tag=f"w3_{k}", name=f"w3_{k}")
                   for k in range(2)]
            for k in range(2):
                nc.scalar.dma_start(out=w2t[k][:], in_=w2b[k * 128:(k + 1) * 128, :])
                nc.scalar.dma_start(out=w3t[k][:], in_=w3b[k * 128:(k + 1) * 128, :])
            w4s = cpool.tile([128, 2 * 128], BF16, tag="w4s")
            nc.sync.dma_start(out=w4s[:], in_=w4b[:])

            b1s = pk[:, 0:2]
            b2s = pk[:, 2:4]
            b3s = pk[:, 4:6]
            b4s = pk[0:1, 6:7]

            lrow = cpool.tile([1, A_PAD], F32, tag="lrow")

            # ---- node gather: nts_all[d, slot] = node_emb[id(slot), d] ---
            nts_all = cpool.tile([128, A_PAD], BF16, tag="nts_all")
            gather_plan = (
                [(0, off, n, 0)
                 for off, n in _gather_chunks(C0, lead=(256, 256, 512))]
                + [(C0, off, n, 1) for off, n in _gather_chunks(C1)])

            if not USE_TGATHER and not USE_DMAT:
                # host-provided identity: keeps the gpsimd queue clear so the
                # Q7 gather-library load starts as early as possible
                ident = cpool.tile([128, 128], BF16, tag="ident")
                nc.sync.dma_start(out=ident[:], in_=identw[:])

            # one-time register loads for the gather index counts
            nregs = {n: nc.gpsimd.to_reg(n) for n in (256, 512, 1024)}

            def emit_gather(gi):
                zone, off, n, grp = gather_plan[gi]
                gsrc = node_emb[0:SPLIT, :] if grp == 0 \
                    else node_emb[SPLIT:N_NODES, :]
                itile = i0 if grp == 0 else i1
                s0 = zone + off
                nreg = nregs[n]
                if USE_TGATHER:
                    nc.gpsimd.dma_gather(
                        out_ap=nts_all[:, s0:s0 + n].unsqueeze(1),
                        in_ap=gsrc,
                        idxs_ap=itile[:, off // 16:(off + n) // 16],
                        num_idxs=n, num_idxs_reg=nreg,
                        elem_size=D, transpose=True, single_packet=False,
                        queue_num=1)
                    return n
                graw = gpool.tile([128, n // 128, D], BF16, tag="graw",
                                  name="graw")
                nc.gpsimd.dma_gather(
                    out_ap=graw[:],
                    in_ap=gsrc,
                    idxs_ap=itile[:, off // 16:(off + n) // 16],
                    num_idxs=n, num_idxs_reg=nreg,
                    elem_size=D, transpose=False, single_packet=False,
                    queue_num=1 + (gi % 8) % 3)
                if USE_DMAT:
                    nc.sync.dma_start_transpose(
                        out=nts_all[:, s0:s0 + n].rearrange(
                            "p (c i) -> p c i", i=128),
                        in_=graw[:].rearrange("p c d -> p (c d)"))
                    return n
                nt_ps = pnt_pool.tile([128, GCHUNK], BF16, space="PSUM",
                                      tag="nt_ps", name="nt_ps")
                for c in range(n // 128):
                    nc.tensor.transpose(
                        out=nt_ps[:, c * 128:(c + 1) * 128],
                        in_=graw[:, c, :], identity=ident[:])
                if gi % 2 == 0:
                    nc.scalar.activation(
                        out=nts_all[:, s0:s0 + n], in_=nt_ps[:, 0:n],
                        func=mybir.ActivationFunctionType.Copy)
                else:
                    nc.vector.tensor_copy(out=nts_all[:, s0:s0 + n],
                                          in_=nt_ps[:, 0:n])
                return n

            def evict_relu(engine, dst, src, bias_ap):
                if engine == "act":
                    nc.scalar.activation(
                        out=dst, in_=src,
                        func=mybir.ActivationFunctionType.Relu, bias=bias_ap)
                else:
                    nc.vector.tensor_scalar(
                        out=dst, in0=src, scalar1=bias_ap, scalar2=0.0,
                        op0=mybir.AluOpType.add, op1=mybir.AluOpType.max)

            # ---- main loop: sweeps of 3 action tiles ----------------------
            # (3-deep tile interleave hides eviction latency at layer
            # boundaries better than 2; PSUM pool ring still covers it)
            SWEEP = 3
            t0s = list(range(0, N_AT, SWEEP))
            out_done = 0                       # cols already DMAed out

            def flush_logits(upto):
                nonlocal out_done
                if upto > out_done:
                    nc.sync.dma_start(out=out_logits[0:1, out_done:upto],
                                      in_=lrow[0:1, out_done:upto])
                    out_done = upto

            gi_next = 0
            covered = 0
            # prime: cover the first two sweeps plus one chunk of lookahead
            while gi_next < len(gather_plan) and \
                    covered < 2 * SWEEP * ATILE + GCHUNK:
                covered += emit_gather(gi_next)
                gi_next += 1
            for si, t0 in enumerate(t0s):
                tiles = list(range(t0, min(t0 + SWEEP, N_AT)))
                sls = [slice(t * ATILE, (t + 1) * ATILE) for t in tiles]
                nt = len(tiles)

                # layer 1: all node matmuls, then all onehot matmuls, so the
                # PE array config (128x128 vs 8x128) switches once per sweep
                h1 = [[hpool.tile([128, ATILE], BF16, tag=f"h1_{j}_{i}",
                                  name=f"h1_{j}_{i}")
                       for j in range(2)] for i in range(nt)]
                hps1 = [[ph_pool.tile([128, ATILE], F32, space="PSUM",
                                      tag="hps", name="hps")
                         for _ in range(nt)] for _ in range(2)]
                for j in range(2):
                    for i in range(nt):
                        nc.tensor.matmul(out=hps1[j][i][:],
                                         lhsT=w1a[:, j * 128:(j + 1) * 128],
                                         rhs=nts_all[:, sls[i]],
                                         start=True, stop=False)
                for j in range(2):
                    for i in range(nt):
                        nc.tensor.matmul(out=hps1[j][i][:],
                                         lhsT=rps[:, j * 128:(j + 1) * 128],
                                         rhs=ohs[:, sls[i]],
                                         start=False, stop=True)
                for j in range(2):
                    for i in range(nt):
                        evict_relu("act" if (i + j) % 2 == 0 else "dve",
                                   h1[i][j][:], hps1[j][i][:], b1s[:, j:j + 1])

                # layers 2 and 3
                hin = h1
                for li, (wt, bs) in enumerate(((w2t, b2s), (w3t, b3s))):
                    hout = [[hpool.tile([128, ATILE], BF16,
                                        tag=f"h{li + 2}_{j}_{i}",
                                        name=f"h{li + 2}_{j}_{i}")
                             for j in range(2)] for i in range(nt)]
                    for j in range(2):
                        hps = [ph_pool.tile([128, ATILE], F32, space="PSUM",
                                            tag="hps", name="hps")
                               for _ in range(nt)]
                        for k in range(2):
                            for i in range(nt):
                                nc.tensor.matmul(
                                    out=hps[i][:],
                                    lhsT=wt[k][:, j * 128:(j + 1) * 128],
                                    rhs=hin[i][k][:],
                                    start=(k == 0), stop=(k == 1))
                        for i in range(nt):
                            evict_relu("act" if (i + j + li) % 2 == 0 else "dve",
                                       hout[i][j][:], hps[i][:], bs[:, j:j + 1])
                    hin = hout

                # layer 4: logits.  w4 replicated across 128 output columns
                # keeps the full 128x128 PE config (no reconfig stall); the
                # eviction reads row 0 of the (identical-row) PSUM result.
                lgs = [ph_pool.tile([128, ATILE], F32, space="PSUM", tag="hps",
                                    name="hps") for _ in range(nt)]
                for k in range(2):
                    for i in range(nt):
                        nc.tensor.matmul(out=lgs[i][:],
                                         lhsT=w4s[:, k * 128:(k + 1) * 128],
                                         rhs=hin[i][k][:],
                                         start=(k == 0), stop=(k == 1))
                for i in range(nt):
                    if i % 2 == 0:
                        nc.scalar.activation(
                            out=lrow[0:1, sls[i]], in_=lgs[i][0:1, :],
                            func=mybir.ActivationFunctionType.Identity,
                            bias=b4s)
                    else:
                        nc.vector.tensor_scalar_add(
                            out=lrow[0:1, sls[i]], in0=lgs[i][0:1, :],
                            scalar1=b4s)
                if si in (2, 4, 6, 7):
                    flush_logits((t0 + SWEEP) * ATILE)
                # gathers for upcoming sweeps — emitted AFTER this sweep's
                # work so their PSUM->SBUF copies queue behind this sweep's
                # evictions on the Scalar/Vector engines
                nxt = min(t0 + 2 * SWEEP, N_AT) * ATILE
                while gi_next < len(gather_plan) and covered < nxt + GCHUNK:
                    covered += emit_gather(gi_next)
                    gi_next += 1

            flush_logits(A_PAD)

    nc.compile()
    return nc


_GRAPH_CACHE = {}


def _get_graph():
    if "g" not in _GRAPH_CACHE:
        _GRAPH_CACHE["g"] = build_graph()
    return _GRAPH_CACHE["g"]


def _wrap_idx(ix):
    """int16 index layout for dma_gather: [16, N/16] column-wrapped,
    replicated 8x down the partitions."""
    w = ix.reshape(-1, 16).T
    return np.ascontiguousarray(np.tile(w, (8, 1)))


def make_in_maps(node_embeddings, region_embeddings, global_context,
                 W1, b1, W2, b2, W3, b3, W4, b4,
                 action_nodes, action_regions):
    """Host-side sharding / marshalling. Returns (in_maps, per-core metas)."""
    W1 = np.asarray(W1, np.float32)
    an = np.asarray(action_nodes).astype(np.int64)
    ar = np.asarray(action_regions).astype(np.int64)
    node_bf16 = np.ascontiguousarray(
        np.asarray(node_embeddings, np.float32).astype(ml_dtypes.bfloat16))
    region_embeddings = np.asarray(region_embeddings, np.float32)

    tail = np.concatenate([
        region_embeddings.reshape(-1),
        np.asarray(global_context, np.float32).reshape(-1)])
    b1c = (np.asarray(b1, np.float32)
           + tail @ W1[2 * D:IN_DIM, :]).astype(np.float32)   # [256]
    rps_np = np.zeros((128, H), ml_dtypes.bfloat16)
    rps_np[0:N_REGIONS] = (region_embeddings @ W1[D:2 * D, :]
                           ).astype(ml_dtypes.bfloat16)
    wa_np = np.ascontiguousarray(W1[0:D, :].astype(ml_dtypes.bfloat16))
    w2b_np = np.ascontiguousarray(
        np.asarray(W2, np.float32).astype(ml_dtypes.bfloat16))
    w3b_np = np.ascontiguousarray(
        np.asarray(W3, np.float32).astype(ml_dtypes.bfloat16))
    w4f = np.asarray(W4, np.float32).reshape(-1)
    w4b_np = np.ascontiguousarray(np.concatenate(
        [np.repeat(w4f[k * 128:(k + 1) * 128, None], 128, axis=1)
         for k in range(2)], axis=1).astype(ml_dtypes.bfloat16))

    pk_base = np.zeros((128, 8), np.float32)
    pk_base[:, 0:2] = b1c.reshape(2, 128).T
    pk_base[:, 2:4] = np.asarray(b2, np.float32).reshape(2, 128).T
    pk_base[:, 4:6] = np.asarray(b3, np.float32).reshape(2, 128).T
    pk_base[0, 6] = np.asarray(b4, np.float32).reshape(-1)[0]

    in_maps, metas = [], []
    for c in range(N_CORES):
        s = c * A_PC
        nodes = an[s:s + A_PC]
        regions = ar[s:s + A_PC]
        grp = (nodes >= SPLIT).astype(np.int8)
        order = np.argsort(grp, kind="stable")      # group0 first, stable
        c0 = int((grp == 0).sum())
        c1 = A_PC - c0
        if c0 > C0 or c1 > C1:
            raise RuntimeError(
                f"core {c}: group sizes {c0}/{c1} exceed capacities {C0}/{C1}")
        sn = nodes[order]
        sr = regions[order]

        ix0 = np.zeros(C0, np.int16)
        ix0[:c0] = sn[:c0].astype(np.int16)
        ix1 = np.zeros(C1, np.int16)
        ix1[:c1] = (sn[c0:] - SPLIT).astype(np.int16)

        slots = np.concatenate([np.arange(c0), C0 + np.arange(c1)])
        oh = np.zeros((128, A_PAD), ml_dtypes.bfloat16)
        oh[sr, slots] = 1.0

        in_maps.append({
            "node_emb": node_bf16,
            "wa": wa_np, "w2b": w2b_np, "w3b": w3b_np,
            "rps_w": rps_np, "w4b": w4b_np,
            "identw": np.eye(128, dtype=ml_dtypes.bfloat16),
            "packed": pk_base,
            "idx0": _wrap_idx(ix0), "idx1": _wrap_idx(ix1),
            "onehot": oh,
        })
        metas.append((order, slots))
    return in_maps, metas


def _unshard(results, metas):
    logits = np.empty(A_FULL, np.float32)
    for c in range(N_CORES):
        order, slots = metas[c]
        lg = np.asarray(results[c]).reshape(-1)[slots]
        logits[c * A_PC:(c + 1) * A_PC][order] = lg
    le = logits.astype(np.float64)
    e = np.exp(le - le.max())
    probs = (e / e.sum()).astype(np.float32)
    return probs, logits


def kernel(**inputs):
    nc = _get_graph()
    in_maps, metas = make_in_maps(**inputs)
    res = bass_utils.run_bass_kernel_spmd(
        nc, in_maps, core_ids=list(range(N_CORES)))
    return _unshard([res.results[c]["out_logits"] for c in range(N_CORES)],
                    metas)


# revision 48
# speedup vs baseline: 1.2411x; 1.0073x over previous
"""Trainium2 Bass kernel for the Actor MLP scorer (gnn_message_passing).

Computation (see reference):
    node_e  = node_embeddings[action_nodes]          # [A, 128] gather
    feats   = [node_e | region_embeddings[action_regions] | const_tail]   # [A, 1427]
    h1..h3  = relu MLP (256 wide), logits = h3 @ W4 + b4                  # [A]
    probs   = softmax(logits) over ALL actions

Strategy (8 NeuronCores, data-parallel over actions):
  - Shard A=100000 actions as 12500/core, sorted by node-id bucket
    (< 32768 vs >= 32768) so the node gather can use the int16-indexed
    DMA-gather ucode over two base-offset views of a bf16 table copy.
    Gathered rows land slot-major and are transposed to [dim, action] on
    the PE; small lead chunks prime the pipeline at startup, and the
    num_idxs registers are hoisted so the Q7 gather-library load starts
    as early as possible.
  - Layer 1 decomposition: feats @ W1 = node_e @ W1[:128]
        + onehot(region) @ (region_embeddings @ W1[128:256])
        + (tail @ W1[256:] + b1)  [host-precomputed constant bias].
    All constant projections (RPS, b1c) are computed on host.  The RPS
    lhsT is zero-padded to K=128 and W4 is replicated across 128 output
    columns so every MLP matmul uses the same 128x128 PE-array config
    (no reconfig stalls); matmul emission is batched per layer.
  - Activations stay transposed ([feature, action]); matmuls bf16 with
    fp32 PSUM; relu+bias evictions split across ScalarE/VectorE.  Gather
    PSUM->SBUF copies are emitted after each sweep's work so they queue
    behind the sweep's evictions.
  - No collectives: each core writes its logits; the global softmax
    normalization (exp/sum/divide) happens on host during unsharding.
"""

import sys

for _p in ("/opt/trn_rl_repo",):
    if _p not in sys.path:
        sys.path.insert(0, _p)

import numpy as np
import ml_dtypes
from concourse import bass, bacc, mybir, tile
from concourse import bass_utils
from concourse.masks import make_identity


# ---------------------------------------------------------------- constants
N_CORES = 8
A_FULL = 100000
N_NODES = 50000
N_REGIONS = 8
D = 128
H = 256
G = 147
IN_DIM = 2 * D + N_REGIONS * D + G          # 1427
F32 = mybir.dt.float32
BF16 = mybir.dt.bfloat16
I16 = mybir.dt.int16

A_PC = A_FULL // N_CORES                    # 12500
SPLIT = 32768                               # int16 index range boundary
C0 = 8704                                   # capacity, node id < 32768
C1 = 4608                                   # capacity, node id >= 32768
A_PAD = C0 + C1                             # 13312 = 26*512
ATILE = 512
N_AT = A_PAD // ATILE                       # 26
GCHUNK = 1024                               # idxs per dma_gather call

USE_TGATHER = False                         # dma_gather transpose mode
USE_DMAT = False                            # xbar DMA transpose (vs PE)


def _gather_chunks(total, lead=()):
    """Chunk a zone; `lead` lets the first chunks be small so the pipeline
    primes quickly at startup."""
    out, off = [], 0
    for n in lead:
        out.append((off, n))
        off += n
    while off < total:
        n = min(GCHUNK, total - off)
        out.append((off, n))
        off += n
    return out


def build_graph():
    nc = bacc.Bacc("TRN2", target_bir_lowering=False, debug=False,
                   num_devices=N_CORES, num_swdge_queues=4)

    # ---- I/O --------------------------------------------------------------
    node_emb = nc.dram_tensor("node_emb", [N_NODES, D], BF16, kind="ExternalInput")
    wa = nc.dram_tensor("wa", [D, H], BF16, kind="ExternalInput")
    w2b = nc.dram_tensor("w2b", [H, H], BF16, kind="ExternalInput")
    w3b = nc.dram_tensor("w3b", [H, H], BF16, kind="ExternalInput")
    # rps padded to 128 rows and w4 replicated across 128 columns so the
    # onehot and logit matmuls use the same full 128x128 PE config as the
    # rest (no PE-array reconfig stalls)
    rps_w = nc.dram_tensor("rps_w", [128, H], BF16, kind="ExternalInput")
    w4b = nc.dram_tensor("w4b", [128, 2 * 128], BF16, kind="ExternalInput")
    identw = nc.dram_tensor("identw", [128, 128], BF16, kind="ExternalInput")
    # cols 0:2 b1c | 2:4 b2 | 4:6 b3 | [0,6] b4
    packed = nc.dram_tensor("packed", [128, 8], F32, kind="ExternalInput")
    idx0 = nc.dram_tensor("idx0", [128, C0 // 16], I16, kind="ExternalInput")
    idx1 = nc.dram_tensor("idx1", [128, C1 // 16], I16, kind="ExternalInput")
    onehot = nc.dram_tensor("onehot", [128, A_PAD], BF16, kind="ExternalInput")

    out_logits = nc.dram_tensor("out_logits", [1, A_PAD], F32, kind="ExternalOutput")

    with tile.TileContext(nc) as tc:
        with (
            tc.tile_pool(name="const", bufs=1) as cpool,
            tc.tile_pool(name="hbuf", bufs=2) as hpool,
            tc.tile_pool(name="graw", bufs=9) as gpool,
            tc.tile_pool(name="pnt", bufs=1, space="PSUM") as pnt_pool,
            tc.tile_pool(name="ph", bufs=7, space="PSUM") as ph_pool,
        ):
            # ---- index loads first: gathers depend on them ---------------
            i0 = cpool.tile([128, C0 // 16], I16, tag="i0")
            nc.sync.dma_start(out=i0[:], in_=idx0[:])
            i1 = cpool.tile([128, C1 // 16], I16, tag="i1")
            nc.sync.dma_start(out=i1[:], in_=idx1[:])

            # ---- constant loads (host pre-cast bf16) ----------------------
            w1a = cpool.tile([128, H], BF16, tag="w1a")
            nc.sync.dma_start(out=w1a[:], in_=wa[:])
            rps = cpool.tile([128, H], BF16, tag="rps")
            nc.sync.dma_start(out=rps[:], in_=rps_w[:])
            pk = cpool.tile([128, 8], F32, tag="pk")
            nc.sync.dma_start(out=pk[:], in_=packed[:])
            # onehot quarters so early action tiles' columns land first
            ohs = cpool.tile([128, A_PAD], BF16, tag="ohs")
            OHQ = A_PAD // 4
            for q in range(4):
                nc.scalar.dma_start(out=ohs[:, q * OHQ:(q + 1) * OHQ],
                                    in_=onehot[:, q * OHQ:(q + 1) * OHQ])
            w2t = [cpool.tile([128, H], BF16, tag=f"w2_{k}", name=f"w2_{k}")
                   for k in range(2)]
            w3t = [cpool.tile([128, H], BF16, tag=f"w3_{k}", name=f"w3_{k}")
                   for k in range(2)]
            for k in range(2):
                nc.scalar.dma_start(out=w2t[k][:], in_=w2b[k * 128:(k + 1) * 128, :])
                nc.scalar.dma_start(out=w3t[k][:], in_=w3b[k * 128:(k + 1) * 128, :])
            w4s = cpool.tile([128, 2 * 128], BF16, tag="w4s")
            nc.sync.dma_start(out=w4s[:], in_=w4b[:])

            b1s = pk[:, 0:2]
            b2s = pk[:, 2:4]
            b3s = pk[:, 4:6]
            b4s = pk[0:1, 6:7]

            lrow = cpool.tile([1, A_PAD], F32, tag="lrow")

            # ---- node gather: nts_all[d, slot] = node_emb[id(slot), d] ---
            nts_all = cpool.tile([128, A_PAD], BF16, tag="nts_all")
            gather_plan = (
                [(0, off, n, 0)
                 for off, n in _gather_chunks(C0, lead=(256, 256, 512))]
                + [(C0, off, n, 1) for off, n in _gather_chunks(C1)])

            if not USE_TGATHER and not USE_DMAT:
                # host-provided identity: keeps the gpsimd queue clear so the
                # Q7 gather-library load starts as early as possible
                ident = cpool.tile([128, 128], BF16, tag="ident")
                nc.sync.dma_start(out=ident[:], in_=identw[:])

            # one-time register loads for the gather index counts
            nregs = {n: nc.gpsimd.to_reg(n) for n in (256, 512, 1024)}

            def emit_gather(gi):
                zone, off, n, grp = gather_plan[gi]
                gsrc = node_emb[0:SPLIT, :] if grp == 0 \
                    else node_emb[SPLIT:N_NODES, :]
                itile = i0 if grp == 0 else i1
                s0 = zone + off
                nreg = nregs[n]
                if USE_TGATHER:
                    nc.gpsimd.dma_gather(
                        out_ap=nts_all[:, s0:s0 + n].unsqueeze(1),
                        in_ap=gsrc,
                        idxs_ap=itile[:, off // 16:(off + n) // 16],
                        num_idxs=n, num_idxs_reg=nreg,
                        elem_size=D, transpose=True, single_packet=False,
                        queue_num=1)
                    return n
                graw = gpool.tile([128, n // 128, D], BF16, tag="graw",
                                  name="graw")
                nc.gpsimd.dma_gather(
                    out_ap=graw[:],
                    in_ap=gsrc,
                    idxs_ap=itile[:, off // 16:(off + n) // 16],
                    num_idxs=n, num_idxs_reg=nreg,
                    elem_size=D, transpose=False, single_packet=False,
                    queue_num=1 + (gi % 8) % 3)
                if USE_DMAT:
                    nc.sync.dma_start_transpose(
                        out=nts_all[:, s0:s0 + n].rearrange(
                            "p (c i) -> p c i", i=128),
                        in_=graw[:].rearrange("p c d -> p (c d)"))
                    return n
                nt_ps = pnt_pool.tile([128, GCHUNK], BF16, space="PSUM",
                                      tag="nt_ps", name="nt_ps")
                for c in range(n // 128):
                    nc.tensor.transpose(
                        out=nt_ps[:, c * 128:(c + 1) * 128],
                        in_=graw[:, c, :], identity=ident[:])
                # split the PSUM->SBUF copy across both eviction engines:
                # halves the queue block and each consuming L1 tile's rhs
                # lies entirely in one half (chunk = 2 tiles)
                h = n // 2
                nc.scalar.activation(
                    out=nts_all[:, s0:s0 + h], in_=nt_ps[:, 0:h],
                    func=mybir.ActivationFunctionType.Copy)
                nc.vector.tensor_copy(out=nts_all[:, s0 + h:s0 + n],
                                      in_=nt_ps[:, h:n])
                return n

            def evict_relu(engine, dst, src, bias_ap):
                if engine == "act":
                    nc.scalar.activation(
                        out=dst, in_=src,
                        func=mybir.ActivationFunctionType.Relu, bias=bias_ap)
                else:
                    nc.vector.tensor_scalar(
                        out=dst, in0=src, scalar1=bias_ap, scalar2=0.0,
                        op0=mybir.AluOpType.add, op1=mybir.AluOpType.max)

            # ---- main loop: sweeps of 3 action tiles ----------------------
            # (3-deep tile interleave hides eviction latency at layer
            # boundaries better than 2; PSUM pool ring still covers it)
            SWEEP = 3
            t0s = list(range(0, N_AT, SWEEP))
            out_done = 0                       # cols already DMAed out

            def flush_logits(upto):
                nonlocal out_done
                if upto > out_done:
                    nc.sync.dma_start(out=out_logits[0:1, out_done:upto],
                                      in_=lrow[0:1, out_done:upto])
                    out_done = upto

            gi_next = 0
            covered = 0
            # prime: cover the first two sweeps plus one chunk of lookahead
            while gi_next < len(gather_plan) and \
                    covered < 2 * SWEEP * ATILE + GCHUNK:
                covered += emit_gather(gi_next)
                gi_next += 1
            for si, t0 in enumerate(t0s):
                tiles = list(range(t0, min(t0 + SWEEP, N_AT)))
                sls = [slice(t * ATILE, (t + 1) * ATILE) for t in tiles]
                nt = len(tiles)

                # layer 1: all node matmuls, then all onehot matmuls, so the
                # PE array config (128x128 vs 8x128) switches once per sweep
                h1 = [[hpool.tile([128, ATILE], BF16, tag=f"h1_{j}_{i}",
                                  name=f"h1_{j}_{i}")
                       for j in range(2)] for i in range(nt)]
                hps1 = [[ph_pool.tile([128, ATILE], F32, space="PSUM",
                                      tag="hps", name="hps")
                         for _ in range(nt)] for _ in range(2)]
                for j in range(2):
                    for i in range(nt):
                        nc.tensor.matmul(out=hps1[j][i][:],
                                         lhsT=w1a[:, j * 128:(j + 1) * 128],
                                         rhs=nts_all[:, sls[i]],
                                         start=True, stop=False)
                for j in range(2):
                    for i in range(nt):
                        nc.tensor.matmul(out=hps1[j][i][:],
                                         lhsT=rps[:, j * 128:(j + 1) * 128],
                                         rhs=ohs[:, sls[i]],
                                         start=False, stop=True)
                for j in range(2):
                    for i in range(nt):
                        evict_relu("act" if (i + j) % 2 == 0 else "dve",
                                   h1[i][j][:], hps1[j][i][:], b1s[:, j:j + 1])

                # layers 2 and 3
                hin = h1
                for li, (wt, bs) in enumerate(((w2t, b2s), (w3t, b3s))):
                    hout = [[hpool.tile([128, ATILE], BF16,
                                        tag=f"h{li + 2}_{j}_{i}",
                                        name=f"h{li + 2}_{j}_{i}")
                             for j in range(2)] for i in range(nt)]
                    for j in range(2):
                        hps = [ph_pool.tile([128, ATILE], F32, space="PSUM",
                                            tag="hps", name="hps")
                               for _ in range(nt)]
                        for k in range(2):
                            for i in range(nt):
                                nc.tensor.matmul(
                                    out=hps[i][:],
                                    lhsT=wt[k][:, j * 128:(j + 1) * 128],
                                    rhs=hin[i][k][:],
                                    start=(k == 0), stop=(k == 1))
                        for i in range(nt):
                            evict_relu("act" if (i + j + li) % 2 == 0 else "dve",
                                       hout[i][j][:], hps[i][:], bs[:, j:j + 1])
                    hin = hout

                # layer 4: logits.  w4 replicated across 128 output columns
                # keeps the full 128x128 PE config (no reconfig stall); the
                # eviction reads row 0 of the (identical-row) PSUM result.
                lgs = [ph_pool.tile([128, ATILE], F32, space="PSUM", tag="hps",
                                    name="hps") for _ in range(nt)]
                for k in range(2):
                    for i in range(nt):
                        nc.tensor.matmul(out=lgs[i][:],
                                         lhsT=w4s[:, k * 128:(k + 1) * 128],
                                         rhs=hin[i][k][:],
                                         start=(k == 0), stop=(k == 1))
                for i in range(nt):
                    if i % 2 == 0:
                        nc.scalar.activation(
                            out=lrow[0:1, sls[i]], in_=lgs[i][0:1, :],
                            func=mybir.ActivationFunctionType.Identity,
                            bias=b4s)
                    else:
                        nc.vector.tensor_scalar_add(
                            out=lrow[0:1, sls[i]], in0=lgs[i][0:1, :],
                            scalar1=b4s)
                if si in (2, 4, 6, 7):
                    flush_logits((t0 + SWEEP) * ATILE)
                # gathers for upcoming sweeps — emitted AFTER this sweep's
                # work so their PSUM->SBUF copies queue behind this sweep's
                # evictions on the Scalar/Vector engines
                nxt = min(t0 + 2 * SWEEP, N_AT) * ATILE
                while gi_next < len(gather_plan) and \
                        covered < nxt + 2 * GCHUNK:
                    covered += emit_gather(gi_next)
                    gi_next += 1

            flush_logits(A_PAD)

    nc.compile()
    return nc


_GRAPH_CACHE = {}


def _get_graph():
    if "g" not in _GRAPH_CACHE:
        _GRAPH_CACHE["g"] = build_graph()
    return _GRAPH_CACHE["g"]


def _wrap_idx(ix):
    """int16 index layout for dma_gather: [16, N/16] column-wrapped,
    replicated 8x down the partitions."""
    w = ix.reshape(-1, 16).T
    return np.ascontiguousarray(np.tile(w, (8, 1)))


def make_in_maps(node_embeddings, region_embeddings, global_context,
                 W1, b1, W2, b2, W3, b3, W4, b4,
                 action_nodes, action_regions):
    """Host-side sharding / marshalling. Returns (in_maps, per-core metas)."""
    W1 = np.asarray(W1, np.float32)
    an = np.asarray(action_nodes).astype(np.int64)
    ar = np.asarray(action_regions).astype(np.int64)
    node_bf16 = np.ascontiguousarray(
        np.asarray(node_embeddings, np.float32).astype(ml_dtypes.bfloat16))
    region_embeddings = np.asarray(region_embeddings, np.float32)

    tail = np.concatenate([
        region_embeddings.reshape(-1),
        np.asarray(global_context, np.float32).reshape(-1)])
    b1c = (np.asarray(b1, np.float32)
           + tail @ W1[2 * D:IN_DIM, :]).astype(np.float32)   # [256]
    rps_np = np.zeros((128, H), ml_dtypes.bfloat16)
    rps_np[0:N_REGIONS] = (region_embeddings @ W1[D:2 * D, :]
                           ).astype(ml_dtypes.bfloat16)
    wa_np = np.ascontiguousarray(W1[0:D, :].astype(ml_dtypes.bfloat16))
    w2b_np = np.ascontiguousarray(
        np.asarray(W2, np.float32).astype(ml_dtypes.bfloat16))
    w3b_np = np.ascontiguousarray(
        np.asarray(W3, np.float32).astype(ml_dtypes.bfloat16))
    w4f = np.asarray(W4, np.float32).reshape(-1)
    w4b_np = np.ascontiguousarray(np.concatenate(
        [np.repeat(w4f[k * 128:(k + 1) * 128, None], 128, axis=1)
         for k in range(2)], axis=1).astype(ml_dtypes.bfloat16))

    pk_base = np.zeros((128, 8), np.float32)
    pk_base[:, 0:2] = b1c.reshape(2, 128).T
    pk_base[:, 2:4] = np.asarray(b2, np.float32).reshape(2, 128).T
    pk_base[:, 4:6] = np.asarray(b3, np.float32).reshape(2, 128).T
    pk_base[0, 6] = np.asarray(b4, np.float32).reshape(-1)[0]

    in_maps, metas = [], []
    for c in range(N_CORES):
        s = c * A_PC
        nodes = an[s:s + A_PC]
        regions = ar[s:s + A_PC]
        grp = (nodes >= SPLIT).astype(np.int8)
        order = np.argsort(grp, kind="stable")      # group0 first, stable
        c0 = int((grp == 0).sum())
        c1 = A_PC - c0
        if c0 > C0 or c1 > C1:
            raise RuntimeError(
                f"core {c}: group sizes {c0}/{c1} exceed capacities {C0}/{C1}")
        sn = nodes[order]
        sr = regions[order]

        ix0 = np.zeros(C0, np.int16)
        ix0[:c0] = sn[:c0].astype(np.int16)
        ix1 = np.zeros(C1, np.int16)
        ix1[:c1] = (sn[c0:] - SPLIT).astype(np.int16)

        slots = np.concatenate([np.arange(c0), C0 + np.arange(c1)])
        oh = np.zeros((128, A_PAD), ml_dtypes.bfloat16)
        oh[sr, slots] = 1.0

        in_maps.append({
            "node_emb": node_bf16,
            "wa": wa_np, "w2b": w2b_np, "w3b": w3b_np,
            "rps_w": rps_np, "w4b": w4b_np,
            "identw": np.eye(128, dtype=ml_dtypes.bfloat16),
            "packed": pk_base,
            "idx0": _wrap_idx(ix0), "idx1": _wrap_idx(ix1),
            "onehot": oh,
        })
        metas.append((order, slots))
    return in_maps, metas


def _unshard(results, metas):
    logits = np.empty(A_FULL, np.float32)
    for c in range(N_CORES):
        order, slots = metas[c]
        lg = np.asarray(results[c]).reshape(-1)[slots]
        logits[c * A_PC:(c + 1) * A_PC][order] = lg
    le = logits.astype(np.float64)
    e = np.exp(le - le.max())
    probs = (e / e.sum()).astype(np.float32)
    return probs, logits


def kernel(**inputs):
    nc = _get_graph()
    in_maps, metas = make_in_maps(**inputs)
    res = bass_utils.run_bass_kernel_spmd(
        nc, in_maps, core_ids=list(range(N_CORES)))
    return _unshard([res.results[c]["out_logits"] for c in range(N_CORES)],
                    metas)
